# revision 35
# baseline (speedup 1.0000x reference)
"""DenoiseNet (retrieval KNN) Trainium2 kernel, v4.1.

Sharding: 8 cores, core c -> batch b = c//2, query-half h = c%2 (64 of the
M=128 query points). Only cross-core step: host sums 8 partial losses.

Key ideas vs the 95.6us v3 baseline (Pool serialized on 44 indirect-DMA
descriptor preps at ~1us each; DVE 65%):
  - KNN1 selection carries the point index in the value: per-512-segment
    top-8 scores are quantized and packed as u32
    (round(relu(s*8192 + (16384-0.75) - |q|^2*8192)) * 512 + local_idx,
    always < 2^24 so the engine-internal f32 conversion is exact).  The 4
    merge rounds then yield global indices by pure ALU decode
    ((pos>>3)<<9 | val&511) on Pool, eliminating v3's DRAM index bounce
    and its 16 fidx descriptor preps (~16.6us of Pool).
  - segment scans (max8 + max_index) read matmul PSUM directly; clean
    scores reduce to block maxima from PSUM too - no score tile is ever
    copied to SBUF (v3 spent 12us of Activation on copies).
  - frame-coord gathers (the remaining 16 one-offset-per-partition
    indirect DMAs) are emitted inside the chunk loop so the in-order Pool
    queue interleaves each prep with chunk compute instead of blocking
    28 preps ahead of it.
  - chunk phase split across engines: cX*fx / cY*fy on Act (per-partition
    scale AP), cZ*fz+cW and the first add and top-4-mask mean on DVE,
    final score add + fcT(PSUM->SBUF subtract) on Pool (Pool stt is
    rejected by the walrus verifier; tensor_tensor works).
  Offline-validated rel err ~1.0e-2 (gate 2e-2): KNN2 block approximation
  (16-pt Morton blocks, top-12 by block max, T=192) contributes ~1e-2,
  packed KNN1 ~6e-4.
"""

import numpy as np

B, N, M, K, C, F, H = 4, 10000, 128, 32, 4, 128, 128
DSM_SIGMA = 0.01
MHALF = M // 2          # 64 queries per core
NPAD1 = 10240           # noisy cloud padded with far points
NH1 = NPAD1 // 2        # 5120 noisy points per half
SEG1 = 1024             # KNN1 segment = two matmul blocks (SBUF-scanned)
NSEG1 = NH1 // SEG1     # 5 segments per half
NC1 = 2 * NSEG1 * 8     # 80 KNN1 candidates per query
PACK_S = 4096.0         # KNN1 pack scale: q = round((2-d2)*4096) <= 2^13
ROWS = MHALF * K        # 2048 (m,k) rows per core
NCHUNK = ROWS // 128    # 16 chunks of 128 rows
MMB = 512               # matmul block (one PSUM bank of fp32)
BLK = 16                # clean-cloud spatial block size
NPAD2 = 10048           # clean cloud padded to 2*16*314
NPH = NPAD2 // 2        # 5024 sorted clean points per half
NBLK = NPAD2 // BLK     # 628 blocks
NBH = NBLK // 2         # 314 per half
NSEL = 10               # blocks gathered per query
TCAND = NSEL * BLK      # 192 clean candidates per query

_compiled = None


def _build():
    import concourse.bacc as bacc
    import concourse.mybir as mybir
    from concourse.tile import TileContext
    from concourse import bass
    from concourse.masks import make_identity

    dt = mybir.dt
    AF = mybir.ActivationFunctionType
    ALU = mybir.AluOpType
    nc = bacc.Bacc("TRN2", target_bir_lowering=False, debug=False, num_devices=8)

    # ---- inputs (per-core shards, host-prepared) ----
    lhsT8_in = nc.dram_tensor("lhsT8", [8, 128], dt.float32, kind="ExternalInput")
    lhsT8r_in = nc.dram_tensor("lhsT8r", [8, 128], dt.float32r, kind="ExternalInput")
    qT_in = nc.dram_tensor("qT", [3, MHALF], dt.float32, kind="ExternalInput")
    qpackb_in = nc.dram_tensor("qpackb", [128, 1], dt.float32, kind="ExternalInput")
    qrow_in = nc.dram_tensor("qrow", [128, 3], dt.float32, kind="ExternalInput")
    noisyT8_in = nc.dram_tensor("noisyT8", [8, NH1], dt.float32, kind="ExternalInput")
    cleanT8_in = nc.dram_tensor("cleanT8", [8, NPH], dt.float32r, kind="ExternalInput")
    pnoisy_in = nc.dram_tensor("pnoisy", [NPAD1, 3], dt.float32, kind="ExternalInput")
    pblk_in = nc.dram_tensor("pblk", [NBLK, 4 * BLK], dt.float32, kind="ExternalInput")
    fW1_in = nc.dram_tensor("fW1", [3, F], dt.float32, kind="ExternalInput")
    fb1_in = nc.dram_tensor("fb1", [F, 1], dt.float32, kind="ExternalInput")
    fW2_in = nc.dram_tensor("fW2", [F, F], dt.float32, kind="ExternalInput")
    fb2_in = nc.dram_tensor("fb2", [F, 1], dt.float32, kind="ExternalInput")
    sW1a_in = nc.dram_tensor("sW1a", [F, H], dt.float32, kind="ExternalInput")
    sW1b_in = nc.dram_tensor("sW1b", [3, H], dt.float32, kind="ExternalInput")
    sb1_in = nc.dram_tensor("sb1", [H, 1], dt.float32, kind="ExternalInput")
    sW2_in = nc.dram_tensor("sW2", [H, H], dt.float32, kind="ExternalInput")
    sb2_in = nc.dram_tensor("sb2", [H, 1], dt.float32, kind="ExternalInput")
    sW3_in = nc.dram_tensor("sW3", [H, 3], dt.float32, kind="ExternalInput")
    sb3_in = nc.dram_tensor("sb3", [3, 1], dt.float32, kind="ExternalInput")

    partial_out = nc.dram_tensor("partial", [128, NCHUNK], dt.float32, kind="ExternalOutput")

    with TileContext(nc) as tc:
        with (
            tc.tile_pool(name="const", bufs=1) as cpool,
            tc.tile_pool(name="work", bufs=3) as wpool,
            tc.tile_pool(name="persist", bufs=1) as ppool,
            tc.tile_pool(name="npsum", bufs=2, space="PSUM") as npsum,
            tc.tile_pool(name="mpsum", bufs=2, space="PSUM") as mpsum,
            tc.tile_pool(name="fpsum", bufs=2, space="PSUM") as fpsum,
            tc.tile_pool(name="gpsum", bufs=2, space="PSUM") as gpsum,
        ):
            # ---- constants / weights resident in SBUF ----
            lhsT8 = cpool.tile([8, 128], dt.float32)
            lhsT8r = cpool.tile([8, 128], dt.float32r)
            qT = cpool.tile([3, MHALF], dt.float32)
            qpackb = cpool.tile([128, 1], dt.float32)
            qrow = cpool.tile([128, 3], dt.float32)
            noisyT8 = cpool.tile([8, NH1], dt.float32)
            cleanT8 = cpool.tile([8, NPH], dt.float32r)
            fW1 = cpool.tile([3, F], dt.float32)
            fb1 = cpool.tile([F, 1], dt.float32)
            fW2 = cpool.tile([F, F], dt.float32)
            fb2 = cpool.tile([F, 1], dt.float32)
            sW1a = cpool.tile([F, H], dt.float32)
            sW1b = cpool.tile([3, H], dt.float32)
            sb1 = cpool.tile([H, 1], dt.float32)
            sW2 = cpool.tile([H, H], dt.float32)
            sb2 = cpool.tile([H, 1], dt.float32)
            sW3 = cpool.tile([H, 3], dt.float32)
            sb3 = cpool.tile([3, 1], dt.float32)
            nc.sync.dma_start(lhsT8r[:], lhsT8r_in[:])
            nc.sync.dma_start(cleanT8[:, 0:NPH // 2], cleanT8_in[:, 0:NPH // 2])
            nc.sync.dma_start(cleanT8[:, NPH // 2:], cleanT8_in[:, NPH // 2:])
            nc.sync.dma_start(lhsT8[:], lhsT8_in[:])
            nc.sync.dma_start(noisyT8[:, 0:NH1 // 2], noisyT8_in[:, 0:NH1 // 2])
            nc.sync.dma_start(noisyT8[:, NH1 // 2:], noisyT8_in[:, NH1 // 2:])
            for t, src in [(qT, qT_in), (qpackb, qpackb_in), (qrow, qrow_in)]:
                nc.sync.dma_start(t[:], src[:])
            ident = cpool.tile([128, 128], dt.float32)
            make_identity(nc, ident[:])
            # u32 decode constants (Pool-side tt operands)
            c3 = cpool.tile([128, 8], dt.uint32)
            c10 = cpool.tile([128, 8], dt.uint32)
            c1023 = cpool.tile([128, 8], dt.uint32)
            nc.vector.memset(c3[:], 3)
            nc.vector.memset(c10[:], 10)
            nc.vector.memset(c1023[:], 1023)

            # ---- PE warmup: ramp the clock before the big matmuls ----
            wups = npsum.tile([128, MMB], dt.float32, tag="nps")
            for _ in range(4):
                nc.tensor.matmul(wups[:, 0:128], ident[:], ident[:], start=True, stop=True)

            # ---- clean scores (fp32r): block-max straight off PSUM ----
            blkmax = ppool.tile([128, NBH], dt.float32)
            for j0 in range(0, NPH, MMB):
                w = min(MMB, NPH - j0)
                ps = npsum.tile([128, MMB], dt.float32, tag="nps")
                nc.tensor.matmul(ps[:, 0:w], lhsT8r[:], cleanT8[:, j0:j0 + w],
                                 start=True, stop=True)
                b0, nb = j0 // BLK, w // BLK
                nc.vector.tensor_reduce(
                    blkmax[:, b0:b0 + nb],
                    ps[:, 0:w].rearrange("p (b k) -> p b k", b=nb, k=BLK),
                    axis=mybir.AxisListType.X, op=ALU.max)

            # ---- top-12 clean blocks per query, gather their records ----
            blkm = ppool.tile([128, NBLK], dt.float32)
            nc.scalar.copy(blkm[0:MHALF, 0:NBH], blkmax[0:MHALF, :])
            nc.sync.dma_start(blkm[0:MHALF, NBH:NBLK], blkmax[MHALF:128, :])
            nc.sync.dma_start(blkm[MHALF:128, 0:NBH], blkmax[0:MHALF, :])
            nc.scalar.copy(blkm[MHALF:128, NBH:NBLK], blkmax[MHALF:128, :])
            blkid = ppool.tile([128, 16], dt.uint32)
            candB = ppool.tile([128, NSEL * BLK * 4], dt.float32)

            def candb_gather(b):
                nc.gpsimd.indirect_dma_start(
                    out=candB[:, 4 * BLK * b:4 * BLK * (b + 1)], out_offset=None,
                    in_=pblk_in[:],
                    in_offset=bass.IndirectOffsetOnAxis(ap=blkid[:, b:b + 1], axis=0),
                )

            for r in range(2):
                bv = wpool.tile([128, 8], dt.float32, tag="bv")
                nc.vector.max(bv[:], blkm[:])
                nc.vector.max_index(blkid[:, 8 * r:8 * r + 8], bv[:], blkm[:])
                if r == 0:
                    nc.vector.match_replace(blkm[:], bv[:], blkm[:], -1e30)
                # issue gathers as soon as this round's block ids exist, so
                # the Pool queue overlaps them with the remaining DVE work
                for b in range(8 * r, min(8 * r + 8, NSEL)):
                    candb_gather(b)

            # ---- noisy scores: segment scans straight off PSUM, u32 pack ----
            cand1v = ppool.tile([128, NC1 // 2], dt.float32)
            cand1i = ppool.tile([128, NC1 // 2], dt.uint32)
            for s in range(NSEG1):
                sseg = wpool.tile([128, SEG1], dt.float32, tag="sseg")
                for h0 in range(2):
                    ps = npsum.tile([128, MMB], dt.float32, tag="nps")
                    j0 = SEG1 * s + MMB * h0
                    nc.tensor.matmul(ps[:], lhsT8[:], noisyT8[:, j0:j0 + MMB],
                                     start=True, stop=True)
                    nc.scalar.copy(sseg[:, MMB * h0:MMB * (h0 + 1)], ps[:])
                nc.vector.max(cand1v[:, 8 * s:8 * s + 8], sseg[:])
                nc.vector.max_index(cand1i[:, 8 * s:8 * s + 8],
                                    cand1v[:, 8 * s:8 * s + 8], sseg[:])
            # pack: q = round(relu(s*4096 + host-bias(|q|^2))) <= 2^13, then
            # packu = q*1024 + local_idx (exact: < 2^24) -- done in two
            # batches so only the last segment sits on the scans->merge chain
            t1u = ppool.tile([128, NC1 // 2], dt.uint32)
            packu = ppool.tile([128, NC1 // 2], dt.uint32)

            def pack_batch(s0, s1):
                sl = slice(8 * s0, 8 * s1)
                nc.scalar.activation(t1u[:, sl], cand1v[:, sl], AF.Relu,
                                     bias=qpackb[:, 0:1], scale=PACK_S)
                nc.vector.scalar_tensor_tensor(packu[:, sl], t1u[:, sl], SEG1,
                                               cand1i[:, sl],
                                               op0=ALU.mult, op1=ALU.add)
            pack_batch(0, NSEG1 - 1)
            pack_batch(NSEG1 - 1, NSEG1)

            # weight DMAs emitted late so they queue behind the
            # latency-critical front DMAs on HWDGE
            for t, src in [(fW1, fW1_in), (fb1, fb1_in),
                           (fW2, fW2_in), (fb2, fb2_in), (sW1a, sW1a_in),
                           (sW1b, sW1b_in), (sb1, sb1_in), (sW2, sW2_in),
                           (sb2, sb2_in), (sW3, sW3_in), (sb3, sb3_in)]:
                nc.sync.dma_start(t[:], src[:])

            # ---- feat MLP (transposed); only needed by the chunk MLP ----
            h1ps = mpsum.tile([F, MHALF], dt.float32, tag="mlp")
            nc.tensor.matmul(h1ps[:], fW1[:], qT[:], start=True, stop=True)
            h1 = ppool.tile([F, MHALF], dt.float32)
            nc.scalar.activation(h1[:], h1ps[:], AF.Relu, bias=fb1[:, 0:1])
            h2ps = mpsum.tile([F, MHALF], dt.float32, tag="mlp")
            nc.tensor.matmul(h2ps[:], fW2[:], h1[:], start=True, stop=True)
            featT = ppool.tile([F, MHALF], dt.float32)
            nc.scalar.activation(featT[:], h2ps[:], AF.Identity, bias=fb2[:, 0:1])
            hfeat2 = ppool.tile([F, 256], dt.float32)
            featT_b = featT[:].unsqueeze(1).to_broadcast([F, 4, MHALF])
            nc.scalar.copy(hfeat2[:].rearrange("p (a b) -> p a b", a=4, b=MHALF), featT_b)

            # ---- merge: top-32 of 160 packed candidates per query.
            # The candidate array is duplicated into both partition halves
            # so the decoded indices land on the right (a,m) rows with cheap
            # same-partition strided copies instead of per-round DMAs. ----
            candm = ppool.tile([128, NC1], dt.uint32)
            NB1 = 8 * (NSEG1 - 1)
            H1 = NC1 // 2
            nc.scalar.copy(candm[0:MHALF, 0:NB1], packu[0:MHALF, 0:NB1])
            nc.scalar.copy(candm[MHALF:128, H1:H1 + NB1], packu[MHALF:128, 0:NB1])
            nc.sync.dma_start(candm[0:MHALF, H1:H1 + NB1], packu[MHALF:128, 0:NB1])
            nc.sync.dma_start(candm[MHALF:128, 0:NB1], packu[0:MHALF, 0:NB1])
            nc.scalar.copy(candm[0:MHALF, NB1:H1], packu[0:MHALF, NB1:H1])
            nc.scalar.copy(candm[MHALF:128, H1 + NB1:NC1], packu[MHALF:128, NB1:H1])
            nc.sync.dma_start(candm[0:MHALF, H1 + NB1:NC1], packu[MHALF:128, NB1:H1])
            nc.sync.dma_start(candm[MHALF:128, NB1:H1], packu[0:MHALF, NB1:H1])

            offsC = ppool.tile([128, NCHUNK], dt.uint32)
            frames_all = ppool.tile([128, NCHUNK, 3], dt.float32)

            for r in range(4):
                csl = slice(4 * r, 4 * r + 4)
                v8 = wpool.tile([128, 8], dt.uint32, tag="v8u")
                nc.vector.max(v8[:], candm[:])
                pos8 = wpool.tile([128, 8], dt.uint32, tag="pos8")
                nc.vector.max_index(pos8[:], v8[:], candm[:])
                if r < 3:
                    nc.vector.match_replace(candm[:], v8[:], candm[:], 0)
                # gidx = ((pos>>3)<<10) | (v8 & 1023)
                seg = wpool.tile([128, 8], dt.uint32, tag="seg")
                nc.vector.tensor_tensor(seg[:], pos8[:], c3[:], op=ALU.logical_shift_right)
                nc.vector.tensor_tensor(seg[:], seg[:], c10[:], op=ALU.logical_shift_left)
                loc = wpool.tile([128, 8], dt.uint32, tag="loc")
                nc.vector.tensor_tensor(loc[:], v8[:], c1023[:], op=ALU.bitwise_and)
                gidx8 = wpool.tile([128, 8], dt.uint32, tag="gidx8")
                nc.vector.tensor_tensor(gidx8[:], seg[:], loc[:], op=ALU.bitwise_or)
                # row (a,m) holds rank 8r+2c+a at chunk col 4r+c: half a
                # takes every other gidx column, same partitions (Act copy)
                g2 = gidx8[:].rearrange("p (c two) -> p c two", two=2)
                for a in range(2):
                    srcv = g2[MHALF * a:MHALF * (a + 1), :, a:a + 1].rearrange(
                        "m c one -> m (c one)")
                    nc.scalar.copy(offsC[MHALF * a:MHALF * (a + 1), csl], srcv)

            loss_acc = ppool.tile([128, NCHUNK], dt.float32)

            # ---- chunks: score vs candidates, top-4 mask mean, score MLP ----
            cB = candB[:].rearrange("p (t c) -> p t c", c=4)
            cX = cB[:, :, 0:1].rearrange("p t one -> p (t one)")
            cY = cB[:, :, 1:2].rearrange("p t one -> p (t one)")
            cZ = cB[:, :, 2:3].rearrange("p t one -> p (t one)")
            cW = cB[:, :, 3:4].rearrange("p t one -> p (t one)")

            gts = {}

            def frames_gather(ci):
                nc.gpsimd.indirect_dma_start(
                    out=frames_all[:, ci, :], out_offset=None, in_=pnoisy_in[:],
                    in_offset=bass.IndirectOffsetOnAxis(ap=offsC[:, ci:ci + 1], axis=0),
                )

            def chunk_pair_select(c0, fcp, prefetch):
                # both chunks of the pair emitted op-by-op interleaved so
                # each in-order engine queue always has an independent op
                # ready while the sibling chunk waits on a cross-engine hop
                cis = (c0, c0 + 1)
                # frame gathers for the NEXT pair: Pool front-runs
                for ci in prefetch:
                    frames_gather(ci)
                scs = {}
                for ci in cis:
                    fx = frames_all[:, ci, 0:1]
                    fy = frames_all[:, ci, 1:2]
                    fz = frames_all[:, ci, 2:3]
                    # score: fused mult+add stt chain, all on DVE - same op
                    # count as any split but zero cross-engine hops
                    t1 = wpool.tile([128, TCAND], dt.float32, tag=f"t1{ci % 2}",
                                    name=f"t1_{ci}")
                    nc.vector.scalar_tensor_tensor(t1[:], cZ, fz, cW,
                                                   op0=ALU.mult, op1=ALU.add)
                    t2 = wpool.tile([128, TCAND], dt.float32, tag=f"t2{ci % 2}",
                                    name=f"t2_{ci}")
                    nc.vector.scalar_tensor_tensor(t2[:], cY, fy, t1[:],
                                                   op0=ALU.mult, op1=ALU.add)
                    sc = wpool.tile([128, TCAND], dt.float32, tag=f"sc{ci % 2}",
                                    name=f"sc_{ci}")
                    nc.vector.scalar_tensor_tensor(sc[:], cX, fx, t2[:],
                                                   op0=ALU.mult, op1=ALU.add)
                    scs[ci] = sc
                v8s = {}
                for ci in cis:
                    v8 = wpool.tile([128, 8], dt.float32, tag=f"v8{ci % 2}")
                    nc.vector.max(v8[:], scs[ci][:])
                    v8s[ci] = v8
                for ci in cis:
                    # fused top-4 mask + masked sum: (sc >= 4th) * coord
                    csum = wpool.tile([128, 3], dt.float32, tag=f"csum{ci % 2}")
                    junk = wpool.tile([128, TCAND], dt.float32, tag=f"junk{ci % 2}")
                    for d, cD in enumerate((cX, cY, cZ)):
                        nc.vector.scalar_tensor_tensor(junk[:], scs[ci][:],
                                                       v8s[ci][:, 3:4], cD,
                                                       op0=ALU.is_ge, op1=ALU.mult,
                                                       accum_out=csum[:, d:d + 1])
                    gt = wpool.tile([128, 3], dt.float32, tag=f"gt{ci % 2}")
                    nc.vector.scalar_tensor_tensor(gt[:], csum[:], 0.125,
                                                   frames_all[:, ci, :],
                                                   op0=ALU.mult, op1=ALU.subtract)
                    gts[ci] = gt
                for ci in cis:
                    # center the frame coords (Pool) then transpose into the
                    # pair slot: fcp holds frames - q directly
                    fcent = wpool.tile([128, 3], dt.float32, tag=f"fcent{ci % 2}")
                    nc.gpsimd.tensor_tensor(fcent[:], frames_all[:, ci, :], qrow[:],
                                            op=ALU.subtract)
                    nc.tensor.transpose(fcp[:, 128 * (ci % 2):128 * (ci % 2) + 128],
                                        fcent[:], ident[:])

            gpTs = {}

            def mlp_pair(j, fcp):
                fcT = wpool.tile([3, 256], dt.float32, tag="fcT")
                nc.scalar.copy(fcT[:], fcp[:])
                m1ps = mpsum.tile([H, 256], dt.float32, tag="mlp")
                nc.tensor.matmul(m1ps[:], sW1a[:], hfeat2[:], start=True, stop=False)
                nc.tensor.matmul(m1ps[:], sW1b[:], fcT[:], start=False, stop=True)
                m1 = wpool.tile([H, 256], dt.float32, tag="m1")
                nc.scalar.activation(m1[:], m1ps[:], AF.Relu, bias=sb1[:, 0:1])
                m2ps = mpsum.tile([H, 256], dt.float32, tag="mlp")
                nc.tensor.matmul(m2ps[:], sW2[:], m1[:], start=True, stop=True)
                m2 = wpool.tile([H, 256], dt.float32, tag="m2")
                nc.scalar.activation(m2[:], m2ps[:], AF.Relu, bias=sb2[:, 0:1])
                gpTps = mpsum.tile([3, 256], dt.float32, tag="mlp")
                nc.tensor.matmul(gpTps[:], sW3[:], m2[:], start=True, stop=True)
                gpT = wpool.tile([3, 256], dt.float32, tag=f"gpT{j % 2}",
                                 name=f"gpT_{j}")
                nc.scalar.activation(gpT[:], gpTps[:], AF.Identity, bias=sb3[:, 0:1])
                gpTs[j] = gpT

            def loss_pair(j):
                # deferred one pair so these MLP-dependent ops never stall
                # the DVE queue between chunk pairs
                gpT = gpTs.pop(j)
                for ci in (2 * j, 2 * j + 1):
                    gpps = gpsum.tile([128, 3], dt.float32, tag="gpps")
                    nc.tensor.transpose(gpps[:], gpT[:, 128 * (ci % 2):128 * (ci % 2) + 128],
                                        ident[0:3, 0:3])
                    gt = gts.pop(ci)
                    diff = wpool.tile([128, 3], dt.float32, tag="diff")
                    nc.vector.tensor_tensor(diff[:], gt[:], gpps[:], op=ALU.subtract)
                    sq = wpool.tile([128, 3], dt.float32, tag="sq")
                    nc.scalar.activation(sq[:], diff[:], AF.Square,
                                         accum_out=loss_acc[:, ci:ci + 1])

            # last two pairs swapped: pair 7's MLP chain starts earlier and
            # pair 6's select/MLP work hides its latency, shrinking the tail
            order = (0, 1, 2, 3, 4, 5, 7, 6)
            frames_gather(0)
            frames_gather(1)
            pending = []
            for idx_j, j in enumerate(order):
                fcp = fpsum.tile([3, 256], dt.float32, tag="fcp")
                nxt = order[idx_j + 1] if idx_j + 1 < len(order) else None
                prefetch = (2 * nxt, 2 * nxt + 1) if nxt is not None else ()
                chunk_pair_select(2 * j, fcp, prefetch)
                mlp_pair(j, fcp)
                pending.append(j)
                if len(pending) > 1:
                    loss_pair(pending.pop(0))
                if j == 5:
                    # ship finished accumulator columns early so the final
                    # DMA+drain tail only covers the last columns
                    nc.sync.dma_start(partial_out[:, 0:8], loss_acc[:, 0:8])
            for j in pending:
                loss_pair(j)

            # ---- ship raw per-chunk accumulators; host does the final sum ----
            nc.sync.dma_start(partial_out[:, 8:NCHUNK], loss_acc[:, 8:NCHUNK])

    nc.finalize()
    return nc


def _get_compiled():
    global _compiled
    if _compiled is None:
        _compiled = _build()
    return _compiled


def _morton_sort(p, bits=6):
    mn, mx = p.min(0), p.max(0)
    g = np.clip(((p - mn) / (mx - mn + 1e-9) * (1 << bits)).astype(np.int64),
                0, (1 << bits) - 1)
    code = np.zeros(len(p), np.int64)
    for b_ in range(bits):
        for d in range(3):
            code |= ((g[:, d] >> b_) & 1) << (3 * b_ + d)
    return np.argsort(code, kind="stable")


def build_in_maps(pcl_noisy, pcl_clean, pnt_idx,
                  feat_W1, feat_b1, feat_W2, feat_b2,
                  score_W1, score_b1, score_W2, score_b2, score_W3, score_b3):
    pcl_noisy = np.asarray(pcl_noisy, dtype=np.float32)
    pcl_clean = np.asarray(pcl_clean, dtype=np.float32)
    idx = np.asarray(pnt_idx).astype(np.int64)

    f32 = np.float32
    w = {
        "fW1": np.ascontiguousarray(feat_W1, dtype=f32),
        "fb1": np.ascontiguousarray(np.asarray(feat_b1, f32).reshape(F, 1)),
        "fW2": np.ascontiguousarray(feat_W2, dtype=f32),
        "fb2": np.ascontiguousarray(np.asarray(feat_b2, f32).reshape(F, 1)),
        "sW1a": np.ascontiguousarray(np.asarray(score_W1, f32)[3:]),
        "sW1b": np.ascontiguousarray(np.asarray(score_W1, f32)[:3]),
        "sb1": np.ascontiguousarray(np.asarray(score_b1, f32).reshape(H, 1)),
        "sW2": np.ascontiguousarray(score_W2, dtype=f32),
        "sb2": np.ascontiguousarray(np.asarray(score_b2, f32).reshape(H, 1)),
        "sW3": np.ascontiguousarray(score_W3, dtype=f32),
        "sb3": np.ascontiguousarray(np.asarray(score_b3, f32).reshape(3, 1)),
    }

    def t8(p):
        nh = p.shape[0] // 2
        v4 = np.concatenate([p.T, -(p * p).sum(1)[None, :]], axis=0)
        return np.ascontiguousarray(
            np.concatenate([v4[:, :nh], v4[:, nh:]], axis=0), f32)

    in_maps = []
    for c in range(8):
        b, h = c // 2, c % 2
        pn = pcl_noisy[b]
        pc = pcl_clean[b]
        q = pn[idx][h * MHALF:(h + 1) * MHALF]          # (64, 3)

        # noisy cloud padded with far sentinels
        pn_pad = np.concatenate([pn, np.full((NPAD1 - N, 3), 1e2, f32)])

        l4 = np.concatenate([2.0 * q.T, np.ones((1, MHALF), f32)], axis=0)  # (4,64)
        lhs8 = np.zeros((8, 128), f32)
        lhs8[0:4, 0:MHALF] = l4
        lhs8[4:8, MHALF:128] = l4

        order = _morton_sort(pc)
        pcs = pc[order]                                  # sorted clean cloud
        pcs = np.concatenate([pcs, np.full((NPAD2 - N, 3), 1e3, f32)])
        pblk = np.concatenate([2.0 * pcs, -(pcs * pcs).sum(1)[:, None]],
                              axis=1).reshape(NBLK, 4 * BLK)

        # pack bias: relu(s*S + bias) = relu(S*(2 - d2) - 0.75)
        qb = (2.0 * PACK_S - 0.75) - (q * q).sum(1) * PACK_S     # (64,)
        qpackb = np.ascontiguousarray(
            np.concatenate([qb, qb])[:, None], f32)              # (128, 1)
        qrow = np.ascontiguousarray(np.concatenate([q, q], axis=0), f32)

        m = dict(w)
        m.update({
            "lhsT8": lhs8,
            "lhsT8r": lhs8.copy(),
            "qT": np.ascontiguousarray(q.T, f32),
            "qpackb": qpackb,
            "qrow": qrow,
            "noisyT8": t8(pn_pad),
            "cleanT8": t8(pcs),
            "pnoisy": np.ascontiguousarray(pn_pad, f32),
            "pblk": np.ascontiguousarray(pblk, f32),
        })
        in_maps.append(m)
    return in_maps


def kernel(**inputs):
    from concourse.bass_utils import run_bass_kernel_spmd

    nc = _get_compiled()
    in_maps = build_in_maps(**inputs)
    res = run_bass_kernel_spmd(nc, in_maps, list(range(8)))
    total = sum(float(res.results[c]["partial"].sum()) for c in range(8))
    loss = total * 0.5 * (1.0 / DSM_SIGMA) / (B * M * K)
    return np.float32(loss)


# revision 36
# speedup vs baseline: 1.0042x; 1.0042x over previous
"""DenoiseNet (retrieval KNN) Trainium2 kernel, v4.1.

Sharding: 8 cores, core c -> batch b = c//2, query-half h = c%2 (64 of the
M=128 query points). Only cross-core step: host sums 8 partial losses.

Key ideas vs the 95.6us v3 baseline (Pool serialized on 44 indirect-DMA
descriptor preps at ~1us each; DVE 65%):
  - KNN1 selection carries the point index in the value: per-512-segment
    top-8 scores are quantized and packed as u32
    (round(relu(s*8192 + (16384-0.75) - |q|^2*8192)) * 512 + local_idx,
    always < 2^24 so the engine-internal f32 conversion is exact).  The 4
    merge rounds then yield global indices by pure ALU decode
    ((pos>>3)<<9 | val&511) on Pool, eliminating v3's DRAM index bounce
    and its 16 fidx descriptor preps (~16.6us of Pool).
  - segment scans (max8 + max_index) read matmul PSUM directly; clean
    scores reduce to block maxima from PSUM too - no score tile is ever
    copied to SBUF (v3 spent 12us of Activation on copies).
  - frame-coord gathers (the remaining 16 one-offset-per-partition
    indirect DMAs) are emitted inside the chunk loop so the in-order Pool
    queue interleaves each prep with chunk compute instead of blocking
    28 preps ahead of it.
  - chunk phase split across engines: cX*fx / cY*fy on Act (per-partition
    scale AP), cZ*fz+cW and the first add and top-4-mask mean on DVE,
    final score add + fcT(PSUM->SBUF subtract) on Pool (Pool stt is
    rejected by the walrus verifier; tensor_tensor works).
  Offline-validated rel err ~1.0e-2 (gate 2e-2): KNN2 block approximation
  (16-pt Morton blocks, top-12 by block max, T=192) contributes ~1e-2,
  packed KNN1 ~6e-4.
"""

import numpy as np

B, N, M, K, C, F, H = 4, 10000, 128, 32, 4, 128, 128
DSM_SIGMA = 0.01
MHALF = M // 2          # 64 queries per core
NPAD1 = 10240           # noisy cloud padded with far points
NH1 = NPAD1 // 2        # 5120 noisy points per half
SEG1 = 1024             # KNN1 segment = two matmul blocks (SBUF-scanned)
NSEG1 = NH1 // SEG1     # 5 segments per half
NC1 = 2 * NSEG1 * 8     # 80 KNN1 candidates per query
PACK_S = 4096.0         # KNN1 pack scale: q = round((2-d2)*4096) <= 2^13
ROWS = MHALF * K        # 2048 (m,k) rows per core
NCHUNK = ROWS // 128    # 16 chunks of 128 rows
MMB = 512               # matmul block (one PSUM bank of fp32)
BLK = 16                # clean-cloud spatial block size
NPAD2 = 10048           # clean cloud padded to 2*16*314
NPH = NPAD2 // 2        # 5024 sorted clean points per half
NBLK = NPAD2 // BLK     # 628 blocks
NBH = NBLK // 2         # 314 per half
NSEL = 10               # blocks gathered per query
TCAND = NSEL * BLK      # 192 clean candidates per query

_compiled = None


def _build():
    import concourse.bacc as bacc
    import concourse.mybir as mybir
    from concourse.tile import TileContext
    from concourse import bass
    from concourse.masks import make_identity

    dt = mybir.dt
    AF = mybir.ActivationFunctionType
    ALU = mybir.AluOpType
    nc = bacc.Bacc("TRN2", target_bir_lowering=False, debug=False, num_devices=8)

    # ---- inputs (per-core shards, host-prepared) ----
    lhsT8_in = nc.dram_tensor("lhsT8", [8, 128], dt.float32, kind="ExternalInput")
    lhsT8r_in = nc.dram_tensor("lhsT8r", [8, 128], dt.float32r, kind="ExternalInput")
    qT_in = nc.dram_tensor("qT", [3, MHALF], dt.float32, kind="ExternalInput")
    qpackb_in = nc.dram_tensor("qpackb", [128, 1], dt.float32, kind="ExternalInput")
    qrow_in = nc.dram_tensor("qrow", [128, 3], dt.float32, kind="ExternalInput")
    noisyT8_in = nc.dram_tensor("noisyT8", [8, NH1], dt.float32, kind="ExternalInput")
    cleanT8_in = nc.dram_tensor("cleanT8", [8, NPH], dt.float32r, kind="ExternalInput")
    pnoisy_in = nc.dram_tensor("pnoisy", [NPAD1, 3], dt.float32, kind="ExternalInput")
    pblk_in = nc.dram_tensor("pblk", [NBLK, 4 * BLK], dt.float32, kind="ExternalInput")
    biota_in = nc.dram_tensor("biota", [128, NBLK], dt.uint32, kind="ExternalInput")
    fW1_in = nc.dram_tensor("fW1", [3, F], dt.float32, kind="ExternalInput")
    fb1_in = nc.dram_tensor("fb1", [F, 1], dt.float32, kind="ExternalInput")
    fW2_in = nc.dram_tensor("fW2", [F, F], dt.float32, kind="ExternalInput")
    fb2_in = nc.dram_tensor("fb2", [F, 1], dt.float32, kind="ExternalInput")
    sW1a_in = nc.dram_tensor("sW1a", [F, H], dt.float32, kind="ExternalInput")
    sW1b_in = nc.dram_tensor("sW1b", [3, H], dt.float32, kind="ExternalInput")
    sb1_in = nc.dram_tensor("sb1", [H, 1], dt.float32, kind="ExternalInput")
    sW2_in = nc.dram_tensor("sW2", [H, H], dt.float32, kind="ExternalInput")
    sb2_in = nc.dram_tensor("sb2", [H, 1], dt.float32, kind="ExternalInput")
    sW3_in = nc.dram_tensor("sW3", [H, 3], dt.float32, kind="ExternalInput")
    sb3_in = nc.dram_tensor("sb3", [3, 1], dt.float32, kind="ExternalInput")

    partial_out = nc.dram_tensor("partial", [128, NCHUNK], dt.float32, kind="ExternalOutput")

    with TileContext(nc) as tc:
        with (
            tc.tile_pool(name="const", bufs=1) as cpool,
            tc.tile_pool(name="work", bufs=3) as wpool,
            tc.tile_pool(name="persist", bufs=1) as ppool,
            tc.tile_pool(name="npsum", bufs=2, space="PSUM") as npsum,
            tc.tile_pool(name="mpsum", bufs=2, space="PSUM") as mpsum,
            tc.tile_pool(name="fpsum", bufs=2, space="PSUM") as fpsum,
            tc.tile_pool(name="gpsum", bufs=2, space="PSUM") as gpsum,
        ):
            # ---- constants / weights resident in SBUF ----
            lhsT8 = cpool.tile([8, 128], dt.float32)
            lhsT8r = cpool.tile([8, 128], dt.float32r)
            qT = cpool.tile([3, MHALF], dt.float32)
            qpackb = cpool.tile([128, 1], dt.float32)
            qrow = cpool.tile([128, 3], dt.float32)
            noisyT8 = cpool.tile([8, NH1], dt.float32)
            cleanT8 = cpool.tile([8, NPH], dt.float32r)
            fW1 = cpool.tile([3, F], dt.float32)
            fb1 = cpool.tile([F, 1], dt.float32)
            fW2 = cpool.tile([F, F], dt.float32)
            fb2 = cpool.tile([F, 1], dt.float32)
            sW1a = cpool.tile([F, H], dt.float32)
            sW1b = cpool.tile([3, H], dt.float32)
            sb1 = cpool.tile([H, 1], dt.float32)
            sW2 = cpool.tile([H, H], dt.float32)
            sb2 = cpool.tile([H, 1], dt.float32)
            sW3 = cpool.tile([H, 3], dt.float32)
            sb3 = cpool.tile([3, 1], dt.float32)
            nc.sync.dma_start(lhsT8r[:], lhsT8r_in[:])
            nc.sync.dma_start(cleanT8[:, 0:NPH // 2], cleanT8_in[:, 0:NPH // 2])
            nc.sync.dma_start(cleanT8[:, NPH // 2:], cleanT8_in[:, NPH // 2:])
            nc.sync.dma_start(lhsT8[:], lhsT8_in[:])
            nc.sync.dma_start(noisyT8[:, 0:NH1 // 2], noisyT8_in[:, 0:NH1 // 2])
            nc.sync.dma_start(noisyT8[:, NH1 // 2:], noisyT8_in[:, NH1 // 2:])
            biota = cpool.tile([128, NBLK], dt.uint32)
            for t, src in [(qT, qT_in), (qpackb, qpackb_in), (qrow, qrow_in),
                           (biota, biota_in)]:
                nc.sync.dma_start(t[:], src[:])
            ident = cpool.tile([128, 128], dt.float32)
            make_identity(nc, ident[:])
            # u32 decode constants (Pool-side tt operands)
            c3 = cpool.tile([128, 8], dt.uint32)
            c10 = cpool.tile([128, 8], dt.uint32)
            c1023 = cpool.tile([128, 8], dt.uint32)
            nc.vector.memset(c3[:], 3)
            nc.vector.memset(c10[:], 10)
            nc.vector.memset(c1023[:], 1023)

            # ---- PE warmup: ramp the clock before the big matmuls ----
            wups = npsum.tile([128, MMB], dt.float32, tag="nps")
            for _ in range(4):
                nc.tensor.matmul(wups[:, 0:128], ident[:], ident[:], start=True, stop=True)

            # ---- clean scores (fp32r): block-max straight off PSUM ----
            blkmax = ppool.tile([128, NBH], dt.float32)
            for j0 in range(0, NPH, MMB):
                w = min(MMB, NPH - j0)
                ps = npsum.tile([128, MMB], dt.float32, tag="nps")
                nc.tensor.matmul(ps[:, 0:w], lhsT8r[:], cleanT8[:, j0:j0 + w],
                                 start=True, stop=True)
                b0, nb = j0 // BLK, w // BLK
                nc.vector.tensor_reduce(
                    blkmax[:, b0:b0 + nb],
                    ps[:, 0:w].rearrange("p (b k) -> p b k", b=nb, k=BLK),
                    axis=mybir.AxisListType.X, op=ALU.max)

            # ---- top-12 clean blocks per query, gather their records ----
            blkm = ppool.tile([128, NBLK], dt.float32)
            nc.scalar.copy(blkm[0:MHALF, 0:NBH], blkmax[0:MHALF, :])
            nc.sync.dma_start(blkm[0:MHALF, NBH:NBLK], blkmax[MHALF:128, :])
            nc.sync.dma_start(blkm[MHALF:128, 0:NBH], blkmax[0:MHALF, :])
            nc.scalar.copy(blkm[MHALF:128, NBH:NBLK], blkmax[MHALF:128, :])
            blkid = ppool.tile([128, 16], dt.uint32)
            candB = ppool.tile([128, NSEL * BLK * 4], dt.float32)

            def candb_gather(b):
                nc.gpsimd.indirect_dma_start(
                    out=candB[:, 4 * BLK * b:4 * BLK * (b + 1)], out_offset=None,
                    in_=pblk_in[:],
                    in_offset=bass.IndirectOffsetOnAxis(ap=blkid[:, b:b + 1], axis=0),
                )

            # pack block ids into the block-max values (same window/bias as
            # the KNN1 pack): top-k then needs no max_index passes at all
            pbu = ppool.tile([128, NBLK], dt.uint32)
            nc.scalar.activation(pbu[:], blkm[:], AF.Relu, bias=qpackb[:, 0:1],
                                 scale=PACK_S)
            pbm = ppool.tile([128, NBLK], dt.uint32)
            nc.vector.scalar_tensor_tensor(pbm[:], pbu[:], SEG1, biota[:],
                                           op0=ALU.mult, op1=ALU.add)
            for r in range(2):
                bv = wpool.tile([128, 8], dt.uint32, tag="bv")
                nc.vector.max(bv[:], pbm[:])
                nc.vector.tensor_tensor(blkid[:, 8 * r:8 * r + 8], bv[:], c1023[:],
                                        op=ALU.bitwise_and)
                if r == 0:
                    nc.vector.match_replace(pbm[:], bv[:], pbm[:], 0)
                # issue gathers as soon as this round's block ids exist, so
                # the Pool queue overlaps them with the remaining DVE work
                for b in range(8 * r, min(8 * r + 8, NSEL)):
                    candb_gather(b)

            # ---- noisy scores: segment scans straight off PSUM, u32 pack ----
            cand1v = ppool.tile([128, NC1 // 2], dt.float32)
            cand1i = ppool.tile([128, NC1 // 2], dt.uint32)
            for s in range(NSEG1):
                sseg = wpool.tile([128, SEG1], dt.float32, tag="sseg")
                for h0 in range(2):
                    ps = npsum.tile([128, MMB], dt.float32, tag="nps")
                    j0 = SEG1 * s + MMB * h0
                    nc.tensor.matmul(ps[:], lhsT8[:], noisyT8[:, j0:j0 + MMB],
                                     start=True, stop=True)
                    nc.scalar.copy(sseg[:, MMB * h0:MMB * (h0 + 1)], ps[:])
                nc.vector.max(cand1v[:, 8 * s:8 * s + 8], sseg[:])
                nc.vector.max_index(cand1i[:, 8 * s:8 * s + 8],
                                    cand1v[:, 8 * s:8 * s + 8], sseg[:])
            # pack: q = round(relu(s*4096 + host-bias(|q|^2))) <= 2^13, then
            # packu = q*1024 + local_idx (exact: < 2^24) -- done in two
            # batches so only the last segment sits on the scans->merge chain
            t1u = ppool.tile([128, NC1 // 2], dt.uint32)
            packu = ppool.tile([128, NC1 // 2], dt.uint32)

            def pack_batch(s0, s1):
                sl = slice(8 * s0, 8 * s1)
                nc.scalar.activation(t1u[:, sl], cand1v[:, sl], AF.Relu,
                                     bias=qpackb[:, 0:1], scale=PACK_S)
                nc.vector.scalar_tensor_tensor(packu[:, sl], t1u[:, sl], SEG1,
                                               cand1i[:, sl],
                                               op0=ALU.mult, op1=ALU.add)
            pack_batch(0, NSEG1 - 1)
            pack_batch(NSEG1 - 1, NSEG1)

            # weight DMAs emitted late so they queue behind the
            # latency-critical front DMAs on HWDGE
            for t, src in [(fW1, fW1_in), (fb1, fb1_in),
                           (fW2, fW2_in), (fb2, fb2_in), (sW1a, sW1a_in),
                           (sW1b, sW1b_in), (sb1, sb1_in), (sW2, sW2_in),
                           (sb2, sb2_in), (sW3, sW3_in), (sb3, sb3_in)]:
                nc.sync.dma_start(t[:], src[:])

            # ---- feat MLP (transposed); only needed by the chunk MLP ----
            h1ps = mpsum.tile([F, MHALF], dt.float32, tag="mlp")
            nc.tensor.matmul(h1ps[:], fW1[:], qT[:], start=True, stop=True)
            h1 = ppool.tile([F, MHALF], dt.float32)
            nc.scalar.activation(h1[:], h1ps[:], AF.Relu, bias=fb1[:, 0:1])
            h2ps = mpsum.tile([F, MHALF], dt.float32, tag="mlp")
            nc.tensor.matmul(h2ps[:], fW2[:], h1[:], start=True, stop=True)
            featT = ppool.tile([F, MHALF], dt.float32)
            nc.scalar.activation(featT[:], h2ps[:], AF.Identity, bias=fb2[:, 0:1])
            hfeat2 = ppool.tile([F, 256], dt.float32)
            featT_b = featT[:].unsqueeze(1).to_broadcast([F, 4, MHALF])
            nc.scalar.copy(hfeat2[:].rearrange("p (a b) -> p a b", a=4, b=MHALF), featT_b)

            # ---- merge: top-32 of 160 packed candidates per query.
            # The candidate array is duplicated into both partition halves
            # so the decoded indices land on the right (a,m) rows with cheap
            # same-partition strided copies instead of per-round DMAs. ----
            candm = ppool.tile([128, NC1], dt.uint32)
            NB1 = 8 * (NSEG1 - 1)
            H1 = NC1 // 2
            nc.scalar.copy(candm[0:MHALF, 0:NB1], packu[0:MHALF, 0:NB1])
            nc.scalar.copy(candm[MHALF:128, H1:H1 + NB1], packu[MHALF:128, 0:NB1])
            nc.sync.dma_start(candm[0:MHALF, H1:H1 + NB1], packu[MHALF:128, 0:NB1])
            nc.sync.dma_start(candm[MHALF:128, 0:NB1], packu[0:MHALF, 0:NB1])
            nc.scalar.copy(candm[0:MHALF, NB1:H1], packu[0:MHALF, NB1:H1])
            nc.scalar.copy(candm[MHALF:128, H1 + NB1:NC1], packu[MHALF:128, NB1:H1])
            nc.sync.dma_start(candm[0:MHALF, H1 + NB1:NC1], packu[MHALF:128, NB1:H1])
            nc.sync.dma_start(candm[MHALF:128, NB1:H1], packu[0:MHALF, NB1:H1])

            offsC = ppool.tile([128, NCHUNK], dt.uint32)
            frames_all = ppool.tile([128, NCHUNK, 3], dt.float32)

            for r in range(4):
                csl = slice(4 * r, 4 * r + 4)
                v8 = wpool.tile([128, 8], dt.uint32, tag="v8u")
                nc.vector.max(v8[:], candm[:])
                pos8 = wpool.tile([128, 8], dt.uint32, tag="pos8")
                nc.vector.max_index(pos8[:], v8[:], candm[:])
                if r < 3:
                    nc.vector.match_replace(candm[:], v8[:], candm[:], 0)
                # gidx = ((pos>>3)<<10) | (v8 & 1023)
                seg = wpool.tile([128, 8], dt.uint32, tag="seg")
                nc.vector.tensor_tensor(seg[:], pos8[:], c3[:], op=ALU.logical_shift_right)
                nc.vector.tensor_tensor(seg[:], seg[:], c10[:], op=ALU.logical_shift_left)
                loc = wpool.tile([128, 8], dt.uint32, tag="loc")
                nc.vector.tensor_tensor(loc[:], v8[:], c1023[:], op=ALU.bitwise_and)
                gidx8 = wpool.tile([128, 8], dt.uint32, tag="gidx8")
                nc.vector.tensor_tensor(gidx8[:], seg[:], loc[:], op=ALU.bitwise_or)
                # row (a,m) holds rank 8r+2c+a at chunk col 4r+c: half a
                # takes every other gidx column, same partitions (Act copy)
                g2 = gidx8[:].rearrange("p (c two) -> p c two", two=2)
                for a in range(2):
                    srcv = g2[MHALF * a:MHALF * (a + 1), :, a:a + 1].rearrange(
                        "m c one -> m (c one)")
                    nc.scalar.copy(offsC[MHALF * a:MHALF * (a + 1), csl], srcv)

            loss_acc = ppool.tile([128, NCHUNK], dt.float32)

            # ---- chunks: score vs candidates, top-4 mask mean, score MLP ----
            cB = candB[:].rearrange("p (t c) -> p t c", c=4)
            cX = cB[:, :, 0:1].rearrange("p t one -> p (t one)")
            cY = cB[:, :, 1:2].rearrange("p t one -> p (t one)")
            cZ = cB[:, :, 2:3].rearrange("p t one -> p (t one)")
            cW = cB[:, :, 3:4].rearrange("p t one -> p (t one)")

            gts = {}

            def frames_gather(ci):
                nc.gpsimd.indirect_dma_start(
                    out=frames_all[:, ci, :], out_offset=None, in_=pnoisy_in[:],
                    in_offset=bass.IndirectOffsetOnAxis(ap=offsC[:, ci:ci + 1], axis=0),
                )

            def chunk_pair_select(c0, fcp, prefetch):
                # both chunks of the pair emitted op-by-op interleaved so
                # each in-order engine queue always has an independent op
                # ready while the sibling chunk waits on a cross-engine hop
                cis = (c0, c0 + 1)
                # frame gathers for the NEXT pair: Pool front-runs
                for ci in prefetch:
                    frames_gather(ci)
                scs = {}
                for ci in cis:
                    fx = frames_all[:, ci, 0:1]
                    fy = frames_all[:, ci, 1:2]
                    fz = frames_all[:, ci, 2:3]
                    # score: fused mult+add stt chain, all on DVE - same op
                    # count as any split but zero cross-engine hops
                    t1 = wpool.tile([128, TCAND], dt.float32, tag=f"t1{ci % 2}",
                                    name=f"t1_{ci}")
                    nc.vector.scalar_tensor_tensor(t1[:], cZ, fz, cW,
                                                   op0=ALU.mult, op1=ALU.add)
                    t2 = wpool.tile([128, TCAND], dt.float32, tag=f"t2{ci % 2}",
                                    name=f"t2_{ci}")
                    nc.vector.scalar_tensor_tensor(t2[:], cY, fy, t1[:],
                                                   op0=ALU.mult, op1=ALU.add)
                    sc = wpool.tile([128, TCAND], dt.float32, tag=f"sc{ci % 2}",
                                    name=f"sc_{ci}")
                    nc.vector.scalar_tensor_tensor(sc[:], cX, fx, t2[:],
                                                   op0=ALU.mult, op1=ALU.add)
                    scs[ci] = sc
                v8s = {}
                for ci in cis:
                    v8 = wpool.tile([128, 8], dt.float32, tag=f"v8{ci % 2}")
                    nc.vector.max(v8[:], scs[ci][:])
                    v8s[ci] = v8
                for ci in cis:
                    # fused top-4 mask + masked sum: (sc >= 4th) * coord
                    csum = wpool.tile([128, 3], dt.float32, tag=f"csum{ci % 2}")
                    junk = wpool.tile([128, TCAND], dt.float32, tag=f"junk{ci % 2}")
                    for d, cD in enumerate((cX, cY, cZ)):
                        nc.vector.scalar_tensor_tensor(junk[:], scs[ci][:],
                                                       v8s[ci][:, 3:4], cD,
                                                       op0=ALU.is_ge, op1=ALU.mult,
                                                       accum_out=csum[:, d:d + 1])
                    gt = wpool.tile([128, 3], dt.float32, tag=f"gt{ci % 2}")
                    nc.vector.scalar_tensor_tensor(gt[:], csum[:], 0.125,
                                                   frames_all[:, ci, :],
                                                   op0=ALU.mult, op1=ALU.subtract)
                    gts[ci] = gt
                for ci in cis:
                    # center the frame coords (Pool) then transpose into the
                    # pair slot: fcp holds frames - q directly
                    fcent = wpool.tile([128, 3], dt.float32, tag=f"fcent{ci % 2}")
                    nc.gpsimd.tensor_tensor(fcent[:], frames_all[:, ci, :], qrow[:],
                                            op=ALU.subtract)
                    nc.tensor.transpose(fcp[:, 128 * (ci % 2):128 * (ci % 2) + 128],
                                        fcent[:], ident[:])

            gpTs = {}

            def mlp_pair(j, fcp):
                fcT = wpool.tile([3, 256], dt.float32, tag="fcT")
                nc.scalar.copy(fcT[:], fcp[:])
                m1ps = mpsum.tile([H, 256], dt.float32, tag="mlp")
                nc.tensor.matmul(m1ps[:], sW1a[:], hfeat2[:], start=True, stop=False)
                nc.tensor.matmul(m1ps[:], sW1b[:], fcT[:], start=False, stop=True)
                m1 = wpool.tile([H, 256], dt.float32, tag="m1")
                nc.scalar.activation(m1[:], m1ps[:], AF.Relu, bias=sb1[:, 0:1])
                m2ps = mpsum.tile([H, 256], dt.float32, tag="mlp")
                nc.tensor.matmul(m2ps[:], sW2[:], m1[:], start=True, stop=True)
                m2 = wpool.tile([H, 256], dt.float32, tag="m2")
                nc.scalar.activation(m2[:], m2ps[:], AF.Relu, bias=sb2[:, 0:1])
                gpTps = mpsum.tile([3, 256], dt.float32, tag="mlp")
                nc.tensor.matmul(gpTps[:], sW3[:], m2[:], start=True, stop=True)
                gpT = wpool.tile([3, 256], dt.float32, tag=f"gpT{j % 2}",
                                 name=f"gpT_{j}")
                nc.scalar.activation(gpT[:], gpTps[:], AF.Identity, bias=sb3[:, 0:1])
                gpTs[j] = gpT

            def loss_pair(j):
                # deferred one pair so these MLP-dependent ops never stall
                # the DVE queue between chunk pairs
                gpT = gpTs.pop(j)
                for ci in (2 * j, 2 * j + 1):
                    gpps = gpsum.tile([128, 3], dt.float32, tag="gpps")
                    nc.tensor.transpose(gpps[:], gpT[:, 128 * (ci % 2):128 * (ci % 2) + 128],
                                        ident[0:3, 0:3])
                    gt = gts.pop(ci)
                    diff = wpool.tile([128, 3], dt.float32, tag="diff")
                    nc.vector.tensor_tensor(diff[:], gt[:], gpps[:], op=ALU.subtract)
                    sq = wpool.tile([128, 3], dt.float32, tag="sq")
                    nc.scalar.activation(sq[:], diff[:], AF.Square,
                                         accum_out=loss_acc[:, ci:ci + 1])

            # last two pairs swapped: pair 7's MLP chain starts earlier and
            # pair 6's select/MLP work hides its latency, shrinking the tail
            order = (0, 1, 2, 3, 4, 5, 7, 6)
            frames_gather(0)
            frames_gather(1)
            pending = []
            for idx_j, j in enumerate(order):
                fcp = fpsum.tile([3, 256], dt.float32, tag="fcp")
                nxt = order[idx_j + 1] if idx_j + 1 < len(order) else None
                prefetch = (2 * nxt, 2 * nxt + 1) if nxt is not None else ()
                chunk_pair_select(2 * j, fcp, prefetch)
                mlp_pair(j, fcp)
                pending.append(j)
                if len(pending) > 1:
                    loss_pair(pending.pop(0))
                if j == 5:
                    # ship finished accumulator columns early so the final
                    # DMA+drain tail only covers the last columns
                    nc.sync.dma_start(partial_out[:, 0:8], loss_acc[:, 0:8])
            for j in pending:
                loss_pair(j)

            # ---- ship raw per-chunk accumulators; host does the final sum ----
            nc.sync.dma_start(partial_out[:, 8:NCHUNK], loss_acc[:, 8:NCHUNK])

    nc.finalize()
    return nc


def _get_compiled():
    global _compiled
    if _compiled is None:
        _compiled = _build()
    return _compiled


def _morton_sort(p, bits=6):
    mn, mx = p.min(0), p.max(0)
    g = np.clip(((p - mn) / (mx - mn + 1e-9) * (1 << bits)).astype(np.int64),
                0, (1 << bits) - 1)
    code = np.zeros(len(p), np.int64)
    for b_ in range(bits):
        for d in range(3):
            code |= ((g[:, d] >> b_) & 1) << (3 * b_ + d)
    return np.argsort(code, kind="stable")


def build_in_maps(pcl_noisy, pcl_clean, pnt_idx,
                  feat_W1, feat_b1, feat_W2, feat_b2,
                  score_W1, score_b1, score_W2, score_b2, score_W3, score_b3):
    pcl_noisy = np.asarray(pcl_noisy, dtype=np.float32)
    pcl_clean = np.asarray(pcl_clean, dtype=np.float32)
    idx = np.asarray(pnt_idx).astype(np.int64)

    f32 = np.float32
    w = {
        "fW1": np.ascontiguousarray(feat_W1, dtype=f32),
        "fb1": np.ascontiguousarray(np.asarray(feat_b1, f32).reshape(F, 1)),
        "fW2": np.ascontiguousarray(feat_W2, dtype=f32),
        "fb2": np.ascontiguousarray(np.asarray(feat_b2, f32).reshape(F, 1)),
        "sW1a": np.ascontiguousarray(np.asarray(score_W1, f32)[3:]),
        "sW1b": np.ascontiguousarray(np.asarray(score_W1, f32)[:3]),
        "sb1": np.ascontiguousarray(np.asarray(score_b1, f32).reshape(H, 1)),
        "sW2": np.ascontiguousarray(score_W2, dtype=f32),
        "sb2": np.ascontiguousarray(np.asarray(score_b2, f32).reshape(H, 1)),
        "sW3": np.ascontiguousarray(score_W3, dtype=f32),
        "sb3": np.ascontiguousarray(np.asarray(score_b3, f32).reshape(3, 1)),
    }

    def t8(p):
        nh = p.shape[0] // 2
        v4 = np.concatenate([p.T, -(p * p).sum(1)[None, :]], axis=0)
        return np.ascontiguousarray(
            np.concatenate([v4[:, :nh], v4[:, nh:]], axis=0), f32)

    in_maps = []
    for c in range(8):
        b, h = c // 2, c % 2
        pn = pcl_noisy[b]
        pc = pcl_clean[b]
        q = pn[idx][h * MHALF:(h + 1) * MHALF]          # (64, 3)

        # noisy cloud padded with far sentinels
        pn_pad = np.concatenate([pn, np.full((NPAD1 - N, 3), 1e2, f32)])

        l4 = np.concatenate([2.0 * q.T, np.ones((1, MHALF), f32)], axis=0)  # (4,64)
        lhs8 = np.zeros((8, 128), f32)
        lhs8[0:4, 0:MHALF] = l4
        lhs8[4:8, MHALF:128] = l4

        order = _morton_sort(pc)
        pcs = pc[order]                                  # sorted clean cloud
        pcs = np.concatenate([pcs, np.full((NPAD2 - N, 3), 1e3, f32)])
        pblk = np.concatenate([2.0 * pcs, -(pcs * pcs).sum(1)[:, None]],
                              axis=1).reshape(NBLK, 4 * BLK)

        # pack bias: relu(s*S + bias) = relu(S*(2 - d2) - 0.75)
        qb = (2.0 * PACK_S - 0.75) - (q * q).sum(1) * PACK_S     # (64,)
        qpackb = np.ascontiguousarray(
            np.concatenate([qb, qb])[:, None], f32)              # (128, 1)
        qrow = np.ascontiguousarray(np.concatenate([q, q], axis=0), f32)
        biota = np.ascontiguousarray(
            np.broadcast_to(np.arange(NBLK, dtype=np.uint32), (128, NBLK)))

        m = dict(w)
        m.update({
            "lhsT8": lhs8,
            "lhsT8r": lhs8.copy(),
            "qT": np.ascontiguousarray(q.T, f32),
            "qpackb": qpackb,
            "qrow": qrow,
            "noisyT8": t8(pn_pad),
            "cleanT8": t8(pcs),
            "pnoisy": np.ascontiguousarray(pn_pad, f32),
            "pblk": np.ascontiguousarray(pblk, f32),
            "biota": biota,
        })
        in_maps.append(m)
    return in_maps


def kernel(**inputs):
    from concourse.bass_utils import run_bass_kernel_spmd

    nc = _get_compiled()
    in_maps = build_in_maps(**inputs)
    res = run_bass_kernel_spmd(nc, in_maps, list(range(8)))
    total = sum(float(res.results[c]["partial"].sum()) for c in range(8))
    loss = total * 0.5 * (1.0 / DSM_SIGMA) / (B * M * K)
    return np.float32(loss)
